# revision 27
# baseline (speedup 1.0000x reference)
"""Causal single-head attention (B=4, S=2048, D=DK=1024) on 8 trn2 NeuronCores.

Sharding: data-parallel over batch x interleaved q-blocks. Core c handles
batch b=c//2, parity p=c%2, owning the 8 q-blocks {2j+p : j in 0..7} (128 rows
each). One uniform SPMD program runs on all 8 cores; per-core differences are
carried entirely by the input data (host-side column permutation + mask tile).

v4 design (vs v2/v3, from trace analysis):
- NO PSUM pool transition: the four D-phase PSUM pools (scores ping-pong,
  transpose staging, P accumulators, out accumulators = 8 banks) are created
  up front and phase G *borrows* their tiles for its 8 accumulators. All
  G->D handoffs become fine-grained per-bank dependencies; the 2-2.5us
  pool-release barrier (and the HAM re-throttle behind it) is gone.
- G h=1 runs in 4 groups of 2 accumulators, each drained right after, in
  bank order (sps, tr, pp, op): every bank is free exactly when its next
  tenant needs it and only ~1us of copy work trails G's last matmul, hidden
  under scores j=1.
- xct SBUF layout is parity-outermost so every xct DMA writes contiguous
  1KB partition lines (v2/v3: 256B scatter): G's first chunk lands ~2.5us
  earlier. The score matmul rhs walks the same tile pair-major via a
  rearranged AP, so score column order (and madd/xc layouts) is unchanged.
- Input DMAs split across queues (wqk on sync / xct on gpsimd), madd
  deferred; N=128 warmup burst into the first G accumulator.
- emit_trp emits the previous group's P matmuls before the attnT copy of
  the new group (no false copy->matmul dep); P halves live in two separate
  SBUF tiles; last tile's out path interleaves matmuls with normalize+DMA
  quarters on alternating queues.

Math per core (W_QK = W_Q W_K^T folded on host):
    G^T = W_QK^T X_q^T                 [dk, 1024]   (q = own 8 blocks)
    S   = G X_ctx^T   (contiguous causal prefix, compact layout)
    A   = softmax(S/32 with -1e9 madd pre-scale)    [fp16]
    P   = A X_ctx                       (fp16 operands, fp32 PSUM)
    out = (P W_V) * rcp                 (then scatter rows back on host)
"""

import numpy as np

B, S, D = 4, 2048, 1024
P = 128               # partitions
NJ = 8                # q-tiles per core
NCORES = 8
MASK_FILL = -1.0e9
JORDER = [1, 2, 3, 4, 5, 6, 7, 0]

_cache = {}


def _build_program():
    from contextlib import ExitStack
    import concourse.bass as bass
    import concourse.bacc as bacc
    import concourse.tile as tile
    import concourse.mybir as mybir
    from concourse import masks

    f32 = mybir.dt.float32
    fp16 = mybir.dt.float16
    Exp = mybir.ActivationFunctionType.Exp
    Copy = mybir.ActivationFunctionType.Copy
    AX = mybir.AxisListType.X
    ts = bass.ts

    nc = bacc.Bacc("TRN2", target_bir_lowering=False, debug=False,
                   enable_asserts=False)

    xct_d = nc.dram_tensor("xct", [D, S], fp16, kind="ExternalInput").ap()
    xc_d = nc.dram_tensor("xc", [S, D], fp16, kind="ExternalInput").ap()
    wqk_d = nc.dram_tensor("wqk", [D, D], fp16, kind="ExternalInput").ap()
    wv_d = nc.dram_tensor("wv", [D, D], fp16, kind="ExternalInput").ap()
    madd_d = nc.dram_tensor("madd", [P, 2 * P], f32, kind="ExternalInput").ap()
    out_d = nc.dram_tensor("out", [NJ * P, D], f32, kind="ExternalOutput").ap()

    # DRAM xct columns come host-packed as [1024 own-block | 1024 other] so
    # with the parity-outermost SBUF layout every DMA partition line is a
    # contiguous 1KB run.
    xct_r = xct_d.rearrange("(c p) (s b k) -> c p s b k",
                            p=P, s=2, k=P)              # [8, 128, 2, 8, 128]
    xct_r2 = xct_d.rearrange("(c p) (s b k) -> p c s b k",
                             p=P, s=2, k=P)             # [128, 8, 2, 8, 128]
    xc_r = xc_d.rearrange("(b p) d -> p b d", p=P)      # [128, 16, 1024]
    wqk_r = wqk_d.rearrange("(c p) n -> c p n", p=P)    # [8, 128, 1024]
    wv_r = wv_d.rearrange("(c p) n -> p c n", p=P)      # [128, 8, 1024]

    with tile.TileContext(nc) as tc, ExitStack() as es:
        # ---- persistent pools -------------------------------------------
        perm = es.enter_context(tc.tile_pool(name="perm", bufs=1))
        xct_sb = perm.tile([P, 8, 2, 8, P], fp16)   # X^T (dc, par, pair, col)
        # G^T split per pass so scores for j<=3 only wait on pass-0 copies
        gt0_sb = perm.tile([P, 8, 512], fp16)       # G^T (dt, q of j 0-3)
        gt1_sb = perm.tile([P, 8, 512], fp16)       # G^T (dt, q of j 4-7)
        xc_sb = perm.tile([P, 16, 1024], fp16)      # X rows (pos, d)
        wv_sb = perm.tile([P, 8, 1024], fp16)
        madd_sb = perm.tile([P, 2 * P], f32)
        ident = perm.tile([P, P], fp16)

        statp = tc.alloc_tile_pool(name="stats", bufs=2)
        earlyp = tc.alloc_tile_pool(name="early", bufs=1)
        workp = tc.alloc_tile_pool(name="work", bufs=2)
        srows = [earlyp.tile([P, 2048], f32, tag=f"srow{i}",
                             name=f"srow{i}") for i in range(2)]
        attns = [earlyp.tile([P, 2048], fp16, tag=f"attn{i}",
                             name=f"attn{i}") for i in range(2)]
        attnT = earlyp.tile([P, 2048], fp16, tag="attnT")
        # P halves in separate tiles: P^T group g waits only its own copy
        p_sbs = [[earlyp.tile([P, 512], fp16, tag=f"p{i}h{h}",
                              name=f"p{i}h{h}") for h in range(2)]
                 for i in range(2)]
        pt_sb = earlyp.tile([P, 1024], fp16, tag="pt")

        # All 8 PSUM banks, allocated once for the whole program. Phase G
        # borrows these same tiles (same tags) for its accumulators.
        spsp = tc.alloc_tile_pool(name="sps", bufs=2, space="PSUM")
        trp = tc.alloc_tile_pool(name="trp", bufs=1, space="PSUM")
        ppp = tc.alloc_tile_pool(name="ppp", bufs=1, space="PSUM")
        opsp = tc.alloc_tile_pool(name="ops", bufs=1, space="PSUM")

        def psum_bank(dt, nm):
            pool, tag = [(spsp, "ps"), (spsp, "ps"), (trp, "tr0"),
                         (trp, "tr1"), (ppp, "pp0"), (ppp, "pp1"),
                         (opsp, "op0"), (opsp, "op1")][dt]
            return pool.tile([P, 512], f32, tag=tag, name=nm)

        # ---- phase G: G^T = (W_QK^T X_q^T) ------------------------------
        # Input DMAs: G inputs first, split across two queues (sync: wqk,
        # 2KB lines; gpsimd: xct halves, 1KB lines), madd deferred.
        with tc.tile_pool(name="wqk", bufs=1) as wqkp:
            wqk_sb = wqkp.tile([P, 8, 1024], fp16)
            # G's first chunk (xct[0] + wqk[0] halves) rides sync's fast
            # hardware DGE (~0.8us ramp vs ~1.9us on gpsimd's software
            # queue); the rest of the xct stream rides the dedicated gpsimd
            # queue. Engine-coupled queues (scalar/vector) must NOT carry
            # DMAs: DMA-issue stalls on recycled semaphores block their
            # copy chains (v5), and out DMAs must not share a queue with
            # inputs (v6).
            nc.sync.dma_start(xct_sb[:, 0, 0, 0:4, :],
                              xct_r[0, :, 0, 0:4, :])
            nc.sync.dma_start(wqk_sb[:, 0, 0:512], wqk_r[0][:, 0:512])
            nc.sync.dma_start(wqk_sb[:, 0, 512:1024], wqk_r[0][:, 512:1024])
            for dc in range(1, 8):
                nc.sync.dma_start(wqk_sb[:, dc, :], wqk_r[dc])
                nc.gpsimd.dma_start(xct_sb[:, dc, 0, 0:4, :],
                                    xct_r[dc, :, 0, 0:4, :])
            for dc in range(8):
                nc.gpsimd.dma_start(xct_sb[:, dc, 0, 4:8, :],
                                    xct_r[dc, :, 0, 4:8, :])
            nc.sync.dma_start(madd_sb[:], madd_d)
            masks.make_identity(nc, ident[:])
            # preload the Exp activation table while scalar is idle (else
            # its 1.3us ACT_TABLE_LOAD lands on the post-G critical chain)
            tblw = statp.tile([P, 1], f32, tag="tblw", name="tblw")
            nc.scalar.activation(tblw[:], attns[0][:, 0:1], Exp)

            # h=0: 8 borrowed accumulators, dc-outer (tracks the DMA
            # stream); warmups write accumulator 0 (garbage, overwritten by
            # the start=True matmul) so PE is busy from the preamble end.
            psl0 = {dt: psum_bank(dt, f"psG{dt}0") for dt in range(8)}
            for _ in range(28):
                nc.tensor.matmul(psl0[0][:, 0:128], attns[0][:, 0:P],
                                 attns[0][:, 0:128])
            for dc in range(8):
                for dt in range(8):
                    nc.tensor.matmul(
                        psl0[dt][:], wqk_sb[:, dc, ts(dt, P)],
                        xct_sb[:, dc, 0, 0:4, :],
                        start=(dc == 0), stop=(dc == 7))
            for dt in range(8):
                nc.scalar.copy(gt0_sb[:, dt, 0:256], psl0[dt][:, 0:256])
                nc.vector.tensor_copy(gt0_sb[:, dt, 256:512],
                                      psl0[dt][:, 256:512])
            # h=1 in 2 groups of 4, in bank order (sps+tr, pp+op): each
            # bank is free (its h=0 copy done) just when its group starts,
            # and only the pp/op-bank copies trail G's last matmul.
            for grp in range(2):
                dts = range(4 * grp, 4 * grp + 4)
                psl1 = {dt: psum_bank(dt, f"psG{dt}1") for dt in dts}
                for dc in range(8):
                    for dt in dts:
                        nc.tensor.matmul(
                            psl1[dt][:], wqk_sb[:, dc, ts(dt, P)],
                            xct_sb[:, dc, 0, 4:8, :],
                            start=(dc == 0), stop=(dc == 7))
                for dt in dts:
                    nc.scalar.copy(gt1_sb[:, dt, 0:256], psl1[dt][:, 0:256])
                    nc.vector.tensor_copy(gt1_sb[:, dt, 256:512],
                                          psl1[dt][:, 256:512])

        # phase-D inputs in first-use order (j order 1,2,..,7,0): other-
        # parity halves first (scores 1-3), then wv/xc for out_1/P_1, then
        # the rest. xct stays on gpsimd, the rest on sync; out DMAs ride
        # gpsimd (dedicated; its software-queue drain overlaps the last
        # tile since the final quarters go out on sync/scalar instead).
        # Late-consumed inputs are batched into few big DMAs to keep the
        # epilogue's per-DMA semaphore-check chain short.
        nc.gpsimd.dma_start(xct_sb[:, :, 1, 0:4, :],
                            xct_r2[:, :, 1, 0:4, :])
        nc.sync.dma_start(wv_sb[:], wv_r)
        nc.sync.dma_start(xc_sb[:, 0:4, :], xc_r[:, 0:4, :])
        nc.gpsimd.dma_start(xct_sb[:, :, 1, 4:8, :],
                            xct_r2[:, :, 1, 4:8, :])
        nc.sync.dma_start(xc_sb[:, 4:16, :], xc_r[:, 4:16, :])

        # ---- phase D: software-pipelined attention ----------------------
        trt = [trp.tile([P, 8, P], fp16, tag=f"tr{i}", name=f"tr{i}")
               for i in range(2)]

        def emit_scores(j):
            """Score matmuls + per-chunk PSUM->srow copies and maxes."""
            srow = srows[j % 2]
            npr = j + 1
            nch = (npr + 1) // 2
            mx = statp.tile([P, 8], f32, tag=f"mx{j % 2}", name=f"mx{j}")
            for ch in range(nch):
                pr = 2 * ch
                cp = min(2, npr - pr)
                w = cp * 256
                off = pr * 256
                ps = spsp.tile([P, 512], f32, tag="ps", name=f"s{j}c{ch}")
                gt = gt0_sb if j < 4 else gt1_sb
                # pair-major walk of the parity-outer xct tile: column
                # order stays [own_pr, other_pr, own_pr+1, other_pr+1]
                for dc in range(8):
                    nc.tensor.matmul(
                        ps[:, :w], gt[:, dc, ts(j % 4, P)],
                        xct_sb[:, dc, :, pr:pr + cp, :].rearrange(
                            "p a b k -> p b a k"),
                        start=(dc == 0), stop=(dc == 7))
                if ch == nch - 1:
                    if w == 512:
                        nc.vector.tensor_copy(srow[:, off:off + 256],
                                              ps[:, 0:256])
                    nc.vector.tensor_add(srow[:, off + w - 256:off + w],
                                         ps[:, w - 256:w], madd_sb[:])
                    nc.vector.reduce_max(mx[:, ch:ch + 1],
                                         srow[:, off:off + w], axis=AX)
                else:
                    nc.vector.tensor_copy(srow[:, off:off + w], ps[:, :w])
                    nc.vector.reduce_max(mx[:, ch:ch + 1], ps[:, :w], axis=AX)
            return mx, nch

        def emit_stats(j, mx, nch):
            """Global (negated, pre-scaled) max — emitted before the next
            tile's score chain so it doesn't queue behind it on DVE."""
            nmx = statp.tile([P, 1], f32, tag=f"nmx{j % 2}", name=f"nmx{j}")
            nc.vector.reduce_max(nmx[:], mx[:, :nch], axis=AX, negate=True)
            nc.vector.tensor_scalar_mul(nmx[:], nmx[:], 1.0 / 32.0)
            return nmx

        def emit_norm(j, op0, op1, rcp):
            """Normalize + store one tile's out projection (non-last tiles;
            the last tile is handled inline in emit_tail)."""
            out_sb = workp.tile([P, 1024], f32, tag="out", name=f"out{j}")
            for op, dh in ((op0, 0), (op1, 512)):
                nc.scalar.activation(out_sb[:, dh:dh + 512], op[:], Copy,
                                     scale=rcp[:])
                nc.gpsimd.dma_start(out_d[ts(j, P), dh:dh + 512],
                                    out_sb[:, dh:dh + 512])

        def emit_exps(j, nch, nmx, pout):
            # pure exps on the scalar queue: no accum_out (the row sum is a
            # single deferred DVE reduce in emit_tail), so the serial scalar
            # chain that paces the big tiles' transposes is exps only
            srow, attn = srows[j % 2], attns[j % 2]
            W = 2 * (j + 1) * P
            for ch in range(nch):
                off = 512 * ch
                w = min(512, W - off)
                nc.scalar.activation(attn[:, off:off + w],
                                     srow[:, off:off + w], Exp,
                                     bias=nmx[:], scale=1.0 / 32.0)
            if pout is not None:
                emit_norm(*pout)

        def emit_trp(j):
            attn = attns[j % 2]
            nk = 2 * (j + 1)

            # A^T via batched PE transposes (groups of 4 = one exp segment).
            # Order per group: transposes -> previous group's P matmuls ->
            # attnT copy, so the P matmuls never (falsely) wait on the new
            # group's copy and the copy runs under them on DVE.
            p_sb = p_sbs[j % 2]
            pp0 = ppp.tile([P, 512], f32, tag="pp0", name="pp0")
            pp1 = ppp.tile([P, 512], f32, tag="pp1", name="pp1")

            def p_mms(g0, gn, pp, dh):
                for c in range(g0, g0 + gn):
                    nc.tensor.matmul(pp[:], attnT[:, ts(c, P)],
                                     xc_sb[:, c, dh:dh + 512],
                                     start=(c == 0), stop=(c == nk - 1))

            groups = [(g0, min(4, nk - g0)) for g0 in range(0, nk, 4)]
            for gi, (g0, gn) in enumerate(groups):
                tr = trt[gi % 2]
                for i in range(gn):
                    nc.tensor.transpose(tr[:, i, :], attn[:, ts(g0 + i, P)],
                                        ident[:])
                if gi > 0:
                    p_mms(*groups[gi - 1], pp0, 0)
                nc.vector.tensor_copy(attnT[:, g0 * P:(g0 + gn) * P],
                                      tr[:, :gn, :])
            # half-outer: finish pp0 first so its PSUM->SBUF copy overlaps
            # half 1's matmuls and the P^T transposes never wait on it
            p_mms(*groups[-1], pp0, 0)
            nc.vector.tensor_copy(p_sb[0][:], pp0[:])
            for g0, gn in groups:
                p_mms(g0, gn, pp1, 512)
            nc.vector.tensor_copy(p_sb[1][:], pp1[:])

        def emit_tail(j):
            """Row-sum + pT + out matmuls, deferred one pipeline stage so
            the next tile's score matmuls hide the P PSUM->SBUF copy
            latency and the sum rides the idle DVE window."""
            W = 2 * (j + 1) * P
            last = j == JORDER[-1]
            sumexp = statp.tile([P, 1], f32, tag=f"sum{j % 2}",
                                name=f"sum{j}")
            nc.vector.reduce_sum(sumexp[:], attns[j % 2][:, 0:W], axis=AX)
            rcp = statp.tile([P, 1], f32, tag=f"rcp{j % 2}", name=f"rcp{j}")
            nc.vector.reciprocal(rcp[:], sumexp[:])
            p_sb = p_sbs[j % 2]
            # P^T via batched transposes (two half-bank groups of 4); group
            # g reads only its own P half tile.
            for gi, g0 in enumerate((0, 4)):
                tr = trt[gi % 2]
                for i in range(4):
                    nc.tensor.transpose(tr[:, i, :], p_sb[gi][:, ts(i, P)],
                                        ident[:])
                nc.vector.tensor_copy(pt_sb[:, g0 * P:(g0 + 4) * P],
                                      tr[:, 0:4, :])
            # out = (P W_V) * rcp — half-outer so half 0's normalize + DMA
            # drain under half 1's matmuls (shrinks the last tile's tail)
            op0 = opsp.tile([P, 512], f32, tag="op0", name="op0")
            op1 = opsp.tile([P, 512], f32, tag="op1", name="op1")
            if not last:
                for op, dh in ((op0, 0), (op1, 512)):
                    for dc in range(8):
                        nc.tensor.matmul(op[:], pt_sb[:, ts(dc, P)],
                                         wv_sb[:, dc, dh:dh + 512],
                                         start=(dc == 0), stop=(dc == 7))
                return (j, op0, op1, rcp)
            # last tile: interleave each half's matmuls with its normalize
            # + store, quarters on alternating DMA queues, so the epilogue
            # starts as early as possible.
            out_sb = workp.tile([P, 1024], f32, tag="out", name=f"out{j}")
            for op, dh in ((op0, 0), (op1, 512)):
                for dc in range(8):
                    nc.tensor.matmul(op[:], pt_sb[:, ts(dc, P)],
                                     wv_sb[:, dc, dh:dh + 512],
                                     start=(dc == 0), stop=(dc == 7))
                # normalize quarters on scalar AND vector in parallel
                nc.scalar.activation(out_sb[:, dh:dh + 256],
                                     op[:, 0:256], Copy, scale=rcp[:])
                nc.vector.tensor_scalar_mul(out_sb[:, dh + 256:dh + 512],
                                            op[:, 256:512], rcp[:])
                nc.sync.dma_start(out_d[ts(j, P), dh:dh + 256],
                                  out_sb[:, dh:dh + 256])
                nc.scalar.dma_start(out_d[ts(j, P), dh + 256:dh + 512],
                                    out_sb[:, dh + 256:dh + 512])
            return None

        pend = emit_scores(JORDER[0])
        pout = None       # tile awaiting normalize+store
        ptail = None      # tile awaiting sum+pT+out
        for idx, j in enumerate(JORDER):
            mx, nch = pend
            nmx = emit_stats(j, mx, nch)
            nxt_pend = emit_scores(JORDER[idx + 1]) if idx < NJ - 1 else None
            emit_exps(j, nch, nmx, pout)
            if idx < NJ - 1:
                pout = emit_tail(*ptail) if ptail is not None else None
                emit_trp(j)
            else:
                # pipeline drain: emit trp(j) first so its transposes +
                # P matmuls hide the previous tile's P PSUM->SBUF copy,
                # and out(j-1) then hides this tile's.
                emit_trp(j)
                pout = emit_tail(*ptail)
            ptail = (j,)
            pend = nxt_pend
        if pout is not None:
            emit_norm(*pout)
        ret = emit_tail(*ptail)
        if ret is not None:
            emit_norm(*ret)

        opsp.release()
        ppp.release()
        trp.release()
        workp.release()
        earlyp.release()
        statp.release()
        spsp.release()

    nc.compile()
    return nc


def _prep_inputs(sequence_repr, W_Q, W_K, W_V, mask):
    """Build the 8 per-core input dicts (host-side slicing/permutation)."""
    wqk = np.ascontiguousarray(W_Q @ W_K.T).astype(np.float16)
    wv = np.ascontiguousarray(W_V).astype(np.float16)
    in_maps = []
    meta = []
    for c in range(NCORES):
        b, par = divmod(c, 2)
        pos_blocks = []
        for j in range(NJ):
            pos_blocks += [2 * j + par, 2 * j + 1 - par]
        rows_perm = np.concatenate(
            [np.arange(g * P, (g + 1) * P) for g in pos_blocks])
        xb = sequence_repr[b]
        # xct columns packed [all own blocks | all other blocks] for
        # contiguous DMA lines; xc rows stay position-interleaved
        halves_perm = np.concatenate(
            [np.arange(g * P, (g + 1) * P)
             for g in pos_blocks[0::2] + pos_blocks[1::2]])
        xct = np.ascontiguousarray(xb.T[:, halves_perm]).astype(np.float16)
        xc = np.ascontiguousarray(xb[rows_perm]).astype(np.float16)
        # j-invariant boundary mask: cols [0:128) = own (diagonal) block,
        # [128:256) = other-parity neighbour (all-masked or all-allowed)
        g0, gb0 = par, 1 - par
        qr0 = slice(g0 * P, g0 * P + P)
        madd = np.empty((P, 2 * P), np.float32)
        madd[:, 0:P] = np.where(mask[b, qr0, g0 * P:(g0 + 1) * P],
                                0.0, MASK_FILL)
        madd[:, P:2 * P] = np.where(mask[b, qr0, gb0 * P:(gb0 + 1) * P],
                                    0.0, MASK_FILL)
        in_maps.append({"xct": xct, "xc": xc, "wqk": wqk, "wv": wv,
                        "madd": madd})
        qrows = np.concatenate(
            [np.arange((2 * j + par) * P, (2 * j + par + 1) * P)
             for j in range(NJ)])
        meta.append((b, qrows))
    return in_maps, meta


def run(sequence_repr, W_Q, W_K, W_V, mask, trace=False):
    from concourse.bass_utils import run_bass_kernel_spmd

    if "nc" not in _cache:
        _cache["nc"] = _build_program()
    nc = _cache["nc"]
    in_maps, meta = _prep_inputs(
        np.asarray(sequence_repr, np.float32), np.asarray(W_Q, np.float32),
        np.asarray(W_K, np.float32), np.asarray(W_V, np.float32),
        np.asarray(mask))
    res = run_bass_kernel_spmd(nc, in_maps, core_ids=list(range(NCORES)),
                               trace=trace)
    out = np.empty((B, S, D), np.float32)
    for c in range(NCORES):
        b, qrows = meta[c]
        out[b, qrows] = res.results[c]["out"]
    return out, res


def kernel(**inputs):
    out, _ = run(**inputs)
    return out


# revision 28
# speedup vs baseline: 1.1639x; 1.1639x over previous
"""Causal single-head attention (B=4, S=2048, D=DK=1024) on 8 trn2 NeuronCores.

Sharding: data-parallel over batch x interleaved q-blocks. Core c handles
batch b=c//2, parity p=c%2, owning the 8 q-blocks {2j+p : j in 0..7} (128 rows
each). One uniform SPMD program runs on all 8 cores; per-core differences are
carried entirely by the input data (host-side column permutation + mask tile).

v4 design (vs v2/v3, from trace analysis):
- NO PSUM pool transition: the four D-phase PSUM pools (scores ping-pong,
  transpose staging, P accumulators, out accumulators = 8 banks) are created
  up front and phase G *borrows* their tiles for its 8 accumulators. All
  G->D handoffs become fine-grained per-bank dependencies; the 2-2.5us
  pool-release barrier (and the HAM re-throttle behind it) is gone.
- G h=1 runs in 4 groups of 2 accumulators, each drained right after, in
  bank order (sps, tr, pp, op): every bank is free exactly when its next
  tenant needs it and only ~1us of copy work trails G's last matmul, hidden
  under scores j=1.
- xct SBUF layout is parity-outermost so every xct DMA writes contiguous
  1KB partition lines (v2/v3: 256B scatter): G's first chunk lands ~2.5us
  earlier. The score matmul rhs walks the same tile pair-major via a
  rearranged AP, so score column order (and madd/xc layouts) is unchanged.
- Input DMAs split across queues (wqk on sync / xct on gpsimd), madd
  deferred; N=128 warmup burst into the first G accumulator.
- emit_trp emits the previous group's P matmuls before the attnT copy of
  the new group (no false copy->matmul dep); P halves live in two separate
  SBUF tiles; last tile's out path interleaves matmuls with normalize+DMA
  quarters on alternating queues.

Math per core (W_QK = W_Q W_K^T folded on host):
    G^T = W_QK^T X_q^T                 [dk, 1024]   (q = own 8 blocks)
    S   = G X_ctx^T   (contiguous causal prefix, compact layout)
    A   = softmax(S/32 with -1e9 madd pre-scale)    [fp16]
    P   = A X_ctx                       (fp16 operands, fp32 PSUM)
    out = (P W_V) * rcp                 (then scatter rows back on host)
"""

import numpy as np

B, S, D = 4, 2048, 1024
P = 128               # partitions
NJ = 8                # q-tiles per core
NCORES = 8
MASK_FILL = -1.0e9
JORDER = [1, 2, 3, 4, 5, 6, 7, 0]

_cache = {}


def _build_program():
    from contextlib import ExitStack
    import concourse.bass as bass
    import concourse.bacc as bacc
    import concourse.tile as tile
    import concourse.mybir as mybir
    from concourse import masks

    f32 = mybir.dt.float32
    fp16 = mybir.dt.float16
    Exp = mybir.ActivationFunctionType.Exp
    Copy = mybir.ActivationFunctionType.Copy
    AX = mybir.AxisListType.X
    ts = bass.ts

    nc = bacc.Bacc("TRN2", target_bir_lowering=False, debug=False,
                   enable_asserts=False)

    xct_d = nc.dram_tensor("xct", [D, S], fp16, kind="ExternalInput").ap()
    xc_d = nc.dram_tensor("xc", [S, D], fp16, kind="ExternalInput").ap()
    wqk_d = nc.dram_tensor("wqk", [D, D], fp16, kind="ExternalInput").ap()
    wv_d = nc.dram_tensor("wv", [D, D], fp16, kind="ExternalInput").ap()
    madd_d = nc.dram_tensor("madd", [P, 2 * P], f32, kind="ExternalInput").ap()
    out_d = nc.dram_tensor("out", [NJ * P, D], f32, kind="ExternalOutput").ap()

    # DRAM xct columns come host-packed as [1024 own-block | 1024 other] so
    # with the parity-outermost SBUF layout every DMA partition line is a
    # contiguous 1KB run.
    xct_r = xct_d.rearrange("(c p) (s b k) -> c p s b k",
                            p=P, s=2, k=P)              # [8, 128, 2, 8, 128]
    xct_r2 = xct_d.rearrange("(c p) (s b k) -> p c s b k",
                             p=P, s=2, k=P)             # [128, 8, 2, 8, 128]
    xc_r = xc_d.rearrange("(b p) d -> p b d", p=P)      # [128, 16, 1024]
    wqk_r = wqk_d.rearrange("(c p) n -> c p n", p=P)    # [8, 128, 1024]
    wv_r = wv_d.rearrange("(c p) n -> p c n", p=P)      # [128, 8, 1024]

    with tile.TileContext(nc) as tc, ExitStack() as es:
        # ---- persistent pools -------------------------------------------
        perm = es.enter_context(tc.tile_pool(name="perm", bufs=1))
        xct_sb = perm.tile([P, 8, 2, 8, P], fp16)   # X^T (dc, par, pair, col)
        # G^T split per pass so scores for j<=3 only wait on pass-0 copies
        gt0_sb = perm.tile([P, 8, 512], fp16)       # G^T (dt, q of j 0-3)
        gt1_sb = perm.tile([P, 8, 512], fp16)       # G^T (dt, q of j 4-7)
        xc_sb = perm.tile([P, 16, 1024], fp16)      # X rows (pos, d)
        wv_sb = perm.tile([P, 8, 1024], fp16)
        madd_sb = perm.tile([P, 2 * P], f32)
        ident = perm.tile([P, P], fp16)

        statp = tc.alloc_tile_pool(name="stats", bufs=2)
        earlyp = tc.alloc_tile_pool(name="early", bufs=1)
        workp = tc.alloc_tile_pool(name="work", bufs=2)
        srows = [earlyp.tile([P, 2048], f32, tag=f"srow{i}",
                             name=f"srow{i}") for i in range(2)]
        attns = [earlyp.tile([P, 2048], fp16, tag=f"attn{i}",
                             name=f"attn{i}") for i in range(2)]
        attnT = earlyp.tile([P, 2048], fp16, tag="attnT")
        # P halves in separate tiles: P^T group g waits only its own copy
        p_sbs = [[earlyp.tile([P, 512], fp16, tag=f"p{i}h{h}",
                              name=f"p{i}h{h}") for h in range(2)]
                 for i in range(2)]
        pt_sb = earlyp.tile([P, 1024], fp16, tag="pt")

        # All 8 PSUM banks, allocated once for the whole program. Phase G
        # borrows these same tiles (same tags) for its accumulators.
        spsp = tc.alloc_tile_pool(name="sps", bufs=2, space="PSUM")
        trp = tc.alloc_tile_pool(name="trp", bufs=1, space="PSUM")
        ppp = tc.alloc_tile_pool(name="ppp", bufs=1, space="PSUM")
        opsp = tc.alloc_tile_pool(name="ops", bufs=1, space="PSUM")

        def psum_bank(dt, nm):
            pool, tag = [(spsp, "ps"), (spsp, "ps"), (trp, "tr0"),
                         (trp, "tr1"), (ppp, "pp0"), (ppp, "pp1"),
                         (opsp, "op0"), (opsp, "op1")][dt]
            return pool.tile([P, 512], f32, tag=tag, name=nm)

        # ---- phase G: G^T = (W_QK^T X_q^T) ------------------------------
        # Input DMAs: G inputs first, split across two queues (sync: wqk,
        # 2KB lines; gpsimd: xct halves, 1KB lines), madd deferred.
        with tc.tile_pool(name="wqk", bufs=1) as wqkp:
            wqk_sb = wqkp.tile([P, 8, 1024], fp16)
            # G's first chunk (xct[0] + wqk[0] halves) rides sync's fast
            # hardware DGE (~0.8us ramp vs ~1.9us on gpsimd's software
            # queue); the rest of the xct stream rides the dedicated gpsimd
            # queue. Engine-coupled queues (scalar/vector) must NOT carry
            # DMAs: DMA-issue stalls on recycled semaphores block their
            # copy chains (v5), and out DMAs must not share a queue with
            # inputs (v6).
            nc.sync.dma_start(xct_sb[:, 0, 0, 0:4, :],
                              xct_r[0, :, 0, 0:4, :])
            nc.sync.dma_start(wqk_sb[:, 0, 0:512], wqk_r[0][:, 0:512])
            nc.sync.dma_start(wqk_sb[:, 0, 512:1024], wqk_r[0][:, 512:1024])
            nc.sync.dma_start(xct_sb[:, 1, 0, 0:4, :],
                              xct_r[1, :, 0, 0:4, :])
            for dc in range(1, 8):
                nc.sync.dma_start(wqk_sb[:, dc, :], wqk_r[dc])
                if dc >= 2:
                    nc.gpsimd.dma_start(xct_sb[:, dc, 0, 0:4, :],
                                        xct_r[dc, :, 0, 0:4, :])
            for dc in range(8):
                nc.gpsimd.dma_start(xct_sb[:, dc, 0, 4:8, :],
                                    xct_r[dc, :, 0, 4:8, :])
            nc.sync.dma_start(madd_sb[:], madd_d)
            masks.make_identity(nc, ident[:])
            # preload the Exp activation table while scalar is idle (else
            # its 1.3us ACT_TABLE_LOAD lands on the post-G critical chain)
            tblw = statp.tile([P, 1], f32, tag="tblw", name="tblw")
            nc.scalar.activation(tblw[:], attns[0][:, 0:1], Exp)

            # h=0: 8 borrowed accumulators, dc-outer (tracks the DMA
            # stream); warmups write accumulator 0 (garbage, overwritten by
            # the start=True matmul) so PE is busy from the preamble end.
            psl0 = {dt: psum_bank(dt, f"psG{dt}0") for dt in range(8)}
            for _ in range(28):
                nc.tensor.matmul(psl0[0][:, 0:128], attns[0][:, 0:P],
                                 attns[0][:, 0:128])
            for dc in range(8):
                for dt in range(8):
                    nc.tensor.matmul(
                        psl0[dt][:], wqk_sb[:, dc, ts(dt, P)],
                        xct_sb[:, dc, 0, 0:4, :],
                        start=(dc == 0), stop=(dc == 7))
            for dt in range(8):
                nc.scalar.copy(gt0_sb[:, dt, 0:256], psl0[dt][:, 0:256])
                nc.vector.tensor_copy(gt0_sb[:, dt, 256:512],
                                      psl0[dt][:, 256:512])
            # h=1 in 2 groups of 4, in bank order (sps+tr, pp+op): each
            # bank is free (its h=0 copy done) just when its group starts,
            # and only the pp/op-bank copies trail G's last matmul.
            for grp in range(2):
                dts = range(4 * grp, 4 * grp + 4)
                psl1 = {dt: psum_bank(dt, f"psG{dt}1") for dt in dts}
                for dc in range(8):
                    for dt in dts:
                        nc.tensor.matmul(
                            psl1[dt][:], wqk_sb[:, dc, ts(dt, P)],
                            xct_sb[:, dc, 0, 4:8, :],
                            start=(dc == 0), stop=(dc == 7))
                for dt in dts:
                    nc.scalar.copy(gt1_sb[:, dt, 0:256], psl1[dt][:, 0:256])
                    nc.vector.tensor_copy(gt1_sb[:, dt, 256:512],
                                          psl1[dt][:, 256:512])

        # phase-D inputs in first-use order (j order 1,2,..,7,0): other-
        # parity halves first (scores 1-3), then wv/xc for out_1/P_1, then
        # the rest. xct stays on gpsimd, the rest on sync; out DMAs ride
        # gpsimd (dedicated; its software-queue drain overlaps the last
        # tile since the final quarters go out on sync/scalar instead).
        # Late-consumed inputs are batched into few big DMAs to keep the
        # epilogue's per-DMA semaphore-check chain short.
        nc.gpsimd.dma_start(xct_sb[:, :, 1, 0:4, :],
                            xct_r2[:, :, 1, 0:4, :])
        nc.sync.dma_start(wv_sb[:], wv_r)
        nc.sync.dma_start(xc_sb[:, 0:4, :], xc_r[:, 0:4, :])
        nc.gpsimd.dma_start(xct_sb[:, :, 1, 4:8, :],
                            xct_r2[:, :, 1, 4:8, :])
        nc.sync.dma_start(xc_sb[:, 4:16, :], xc_r[:, 4:16, :])

        # ---- phase D: software-pipelined attention ----------------------
        trt = [trp.tile([P, 8, P], fp16, tag=f"tr{i}", name=f"tr{i}")
               for i in range(2)]

        def emit_scores(j):
            """Score matmuls + per-chunk PSUM->srow copies and maxes."""
            srow = srows[j % 2]
            npr = j + 1
            nch = (npr + 1) // 2
            mx = statp.tile([P, 8], f32, tag=f"mx{j % 2}", name=f"mx{j}")
            for ch in range(nch):
                pr = 2 * ch
                cp = min(2, npr - pr)
                w = cp * 256
                off = pr * 256
                ps = spsp.tile([P, 512], f32, tag="ps", name=f"s{j}c{ch}")
                gt = gt0_sb if j < 4 else gt1_sb
                # pair-major walk of the parity-outer xct tile: column
                # order stays [own_pr, other_pr, own_pr+1, other_pr+1]
                for dc in range(8):
                    nc.tensor.matmul(
                        ps[:, :w], gt[:, dc, ts(j % 4, P)],
                        xct_sb[:, dc, :, pr:pr + cp, :].rearrange(
                            "p a b k -> p b a k"),
                        start=(dc == 0), stop=(dc == 7))
                if ch == nch - 1:
                    if w == 512:
                        nc.vector.tensor_copy(srow[:, off:off + 256],
                                              ps[:, 0:256])
                    nc.vector.tensor_add(srow[:, off + w - 256:off + w],
                                         ps[:, w - 256:w], madd_sb[:])
                    nc.vector.reduce_max(mx[:, ch:ch + 1],
                                         srow[:, off:off + w], axis=AX)
                else:
                    nc.vector.tensor_copy(srow[:, off:off + w], ps[:, :w])
                    nc.vector.reduce_max(mx[:, ch:ch + 1], ps[:, :w], axis=AX)
            return mx, nch

        def emit_stats(j, mx, nch):
            """Global (negated, pre-scaled) max — emitted before the next
            tile's score chain so it doesn't queue behind it on DVE."""
            nmx = statp.tile([P, 1], f32, tag=f"nmx{j % 2}", name=f"nmx{j}")
            nc.vector.reduce_max(nmx[:], mx[:, :nch], axis=AX, negate=True)
            nc.vector.tensor_scalar_mul(nmx[:], nmx[:], 1.0 / 32.0)
            return nmx

        def emit_norm(j, op0, op1, rcp):
            """Normalize + store one tile's out projection (non-last tiles;
            the last tile is handled inline in emit_tail)."""
            out_sb = workp.tile([P, 1024], f32, tag="out", name=f"out{j}")
            for op, dh in ((op0, 0), (op1, 512)):
                nc.scalar.activation(out_sb[:, dh:dh + 512], op[:], Copy,
                                     scale=rcp[:])
                nc.gpsimd.dma_start(out_d[ts(j, P), dh:dh + 512],
                                    out_sb[:, dh:dh + 512])

        def emit_exps(j, nch, nmx, pout):
            # pure exps on the scalar queue: no accum_out (the row sum is a
            # single deferred DVE reduce in emit_tail), so the serial scalar
            # chain that paces the big tiles' transposes is exps only
            srow, attn = srows[j % 2], attns[j % 2]
            W = 2 * (j + 1) * P
            for ch in range(nch):
                off = 512 * ch
                w = min(512, W - off)
                nc.scalar.activation(attn[:, off:off + w],
                                     srow[:, off:off + w], Exp,
                                     bias=nmx[:], scale=1.0 / 32.0)
            if pout is not None:
                emit_norm(*pout)

        def emit_trp(j):
            attn = attns[j % 2]
            nk = 2 * (j + 1)

            # A^T via batched PE transposes (groups of 4 = one exp segment).
            # Order per group: transposes -> previous group's P matmuls ->
            # attnT copy, so the P matmuls never (falsely) wait on the new
            # group's copy and the copy runs under them on DVE.
            p_sb = p_sbs[j % 2]
            pp0 = ppp.tile([P, 512], f32, tag="pp0", name="pp0")
            pp1 = ppp.tile([P, 512], f32, tag="pp1", name="pp1")

            def p_mms(g0, gn, pp, dh):
                for c in range(g0, g0 + gn):
                    nc.tensor.matmul(pp[:], attnT[:, ts(c, P)],
                                     xc_sb[:, c, dh:dh + 512],
                                     start=(c == 0), stop=(c == nk - 1))

            groups = [(g0, min(4, nk - g0)) for g0 in range(0, nk, 4)]
            for gi, (g0, gn) in enumerate(groups):
                tr = trt[gi % 2]
                for i in range(gn):
                    nc.tensor.transpose(tr[:, i, :], attn[:, ts(g0 + i, P)],
                                        ident[:])
                if gi > 0:
                    p_mms(*groups[gi - 1], pp0, 0)
                nc.vector.tensor_copy(attnT[:, g0 * P:(g0 + gn) * P],
                                      tr[:, :gn, :])
            # half-outer: finish pp0 first so its PSUM->SBUF copy overlaps
            # half 1's matmuls and the P^T transposes never wait on it
            p_mms(*groups[-1], pp0, 0)
            nc.vector.tensor_copy(p_sb[0][:], pp0[:])
            for g0, gn in groups:
                p_mms(g0, gn, pp1, 512)
            nc.vector.tensor_copy(p_sb[1][:], pp1[:])

        def emit_tail(j):
            """Row-sum + pT + out matmuls, deferred one pipeline stage so
            the next tile's score matmuls hide the P PSUM->SBUF copy
            latency and the sum rides the idle DVE window."""
            W = 2 * (j + 1) * P
            last = j == JORDER[-1]
            sumexp = statp.tile([P, 1], f32, tag=f"sum{j % 2}",
                                name=f"sum{j}")
            nc.vector.reduce_sum(sumexp[:], attns[j % 2][:, 0:W], axis=AX)
            rcp = statp.tile([P, 1], f32, tag=f"rcp{j % 2}", name=f"rcp{j}")
            nc.vector.reciprocal(rcp[:], sumexp[:])
            p_sb = p_sbs[j % 2]
            # P^T via batched transposes (two half-bank groups of 4); group
            # g reads only its own P half tile.
            for gi, g0 in enumerate((0, 4)):
                tr = trt[gi % 2]
                for i in range(4):
                    nc.tensor.transpose(tr[:, i, :], p_sb[gi][:, ts(i, P)],
                                        ident[:])
                nc.vector.tensor_copy(pt_sb[:, g0 * P:(g0 + 4) * P],
                                      tr[:, 0:4, :])
            # out = (P W_V) * rcp — half-outer so half 0's normalize + DMA
            # drain under half 1's matmuls (shrinks the last tile's tail)
            op0 = opsp.tile([P, 512], f32, tag="op0", name="op0")
            op1 = opsp.tile([P, 512], f32, tag="op1", name="op1")
            if not last:
                for op, dh in ((op0, 0), (op1, 512)):
                    for dc in range(8):
                        nc.tensor.matmul(op[:], pt_sb[:, ts(dc, P)],
                                         wv_sb[:, dc, dh:dh + 512],
                                         start=(dc == 0), stop=(dc == 7))
                return (j, op0, op1, rcp)
            # last tile: interleave each half's matmuls with its normalize
            # + store, quarters on alternating DMA queues, so the epilogue
            # starts as early as possible.
            out_sb = workp.tile([P, 1024], f32, tag="out", name=f"out{j}")
            for op, dh in ((op0, 0), (op1, 512)):
                for dc in range(8):
                    nc.tensor.matmul(op[:], pt_sb[:, ts(dc, P)],
                                     wv_sb[:, dc, dh:dh + 512],
                                     start=(dc == 0), stop=(dc == 7))
                # normalize quarters on scalar AND vector in parallel
                nc.scalar.activation(out_sb[:, dh:dh + 256],
                                     op[:, 0:256], Copy, scale=rcp[:])
                nc.vector.tensor_scalar_mul(out_sb[:, dh + 256:dh + 512],
                                            op[:, 256:512], rcp[:])
                nc.sync.dma_start(out_d[ts(j, P), dh:dh + 256],
                                  out_sb[:, dh:dh + 256])
                nc.scalar.dma_start(out_d[ts(j, P), dh + 256:dh + 512],
                                    out_sb[:, dh + 256:dh + 512])
            return None

        pend = emit_scores(JORDER[0])
        pout = None       # tile awaiting normalize+store
        ptail = None      # tile awaiting sum+pT+out
        for idx, j in enumerate(JORDER):
            mx, nch = pend
            nmx = emit_stats(j, mx, nch)
            nxt_pend = emit_scores(JORDER[idx + 1]) if idx < NJ - 1 else None
            emit_exps(j, nch, nmx, pout)
            if idx < NJ - 1:
                pout = emit_tail(*ptail) if ptail is not None else None
                emit_trp(j)
            else:
                # pipeline drain: emit trp(j) first so its transposes +
                # P matmuls hide the previous tile's P PSUM->SBUF copy,
                # and out(j-1) then hides this tile's.
                emit_trp(j)
                pout = emit_tail(*ptail)
            ptail = (j,)
            pend = nxt_pend
        if pout is not None:
            emit_norm(*pout)
        ret = emit_tail(*ptail)
        if ret is not None:
            emit_norm(*ret)

        opsp.release()
        ppp.release()
        trp.release()
        workp.release()
        earlyp.release()
        statp.release()
        spsp.release()

    nc.compile()
    return nc


def _prep_inputs(sequence_repr, W_Q, W_K, W_V, mask):
    """Build the 8 per-core input dicts (host-side slicing/permutation)."""
    wqk = np.ascontiguousarray(W_Q @ W_K.T).astype(np.float16)
    wv = np.ascontiguousarray(W_V).astype(np.float16)
    in_maps = []
    meta = []
    for c in range(NCORES):
        b, par = divmod(c, 2)
        pos_blocks = []
        for j in range(NJ):
            pos_blocks += [2 * j + par, 2 * j + 1 - par]
        rows_perm = np.concatenate(
            [np.arange(g * P, (g + 1) * P) for g in pos_blocks])
        xb = sequence_repr[b]
        # xct columns packed [all own blocks | all other blocks] for
        # contiguous DMA lines; xc rows stay position-interleaved
        halves_perm = np.concatenate(
            [np.arange(g * P, (g + 1) * P)
             for g in pos_blocks[0::2] + pos_blocks[1::2]])
        xct = np.ascontiguousarray(xb.T[:, halves_perm]).astype(np.float16)
        xc = np.ascontiguousarray(xb[rows_perm]).astype(np.float16)
        # j-invariant boundary mask: cols [0:128) = own (diagonal) block,
        # [128:256) = other-parity neighbour (all-masked or all-allowed)
        g0, gb0 = par, 1 - par
        qr0 = slice(g0 * P, g0 * P + P)
        madd = np.empty((P, 2 * P), np.float32)
        madd[:, 0:P] = np.where(mask[b, qr0, g0 * P:(g0 + 1) * P],
                                0.0, MASK_FILL)
        madd[:, P:2 * P] = np.where(mask[b, qr0, gb0 * P:(gb0 + 1) * P],
                                    0.0, MASK_FILL)
        in_maps.append({"xct": xct, "xc": xc, "wqk": wqk, "wv": wv,
                        "madd": madd})
        qrows = np.concatenate(
            [np.arange((2 * j + par) * P, (2 * j + par + 1) * P)
             for j in range(NJ)])
        meta.append((b, qrows))
    return in_maps, meta


def run(sequence_repr, W_Q, W_K, W_V, mask, trace=False):
    from concourse.bass_utils import run_bass_kernel_spmd

    if "nc" not in _cache:
        _cache["nc"] = _build_program()
    nc = _cache["nc"]
    in_maps, meta = _prep_inputs(
        np.asarray(sequence_repr, np.float32), np.asarray(W_Q, np.float32),
        np.asarray(W_K, np.float32), np.asarray(W_V, np.float32),
        np.asarray(mask))
    res = run_bass_kernel_spmd(nc, in_maps, core_ids=list(range(NCORES)),
                               trace=trace)
    out = np.empty((B, S, D), np.float32)
    for c in range(NCORES):
        b, qrows = meta[c]
        out[b, qrows] = res.results[c]["out"]
    return out, res


def kernel(**inputs):
    out, _ = run(**inputs)
    return out
